# revision 9
# baseline (speedup 1.0000x reference)
"""Max-plus layer (y[b,i] = max_j(x[b,j] + a[i,j]) + bias[i]) on 8 TRN2 cores.

Strategy — tensor-parallel over out_features (64 per core), with a
three-engine PSUM pipeline per (feature, batch-tile) pair so the DVE
only does the irreducible max-reduce pass:

  1. host: fold bias into a (a' = a + bias), split a' EXACTLY into
     three bf16 planes (hi + mid + lo == a' bitwise in fp32).
  2. ScalarE copies the x batch-tile [128, 512] into a PSUM bank
     (bit-exact fp32 copy; the bank's has_written bits were set once by
     a zero bf16 matmul, so later matmuls accumulate instead of
     overwriting — verified on HW).
  3. TensorE adds the broadcast a'-row in ONE accumulating K=3 bf16
     matmul (ones^T[3,128] @ planes[3,512]); the systolic fp32
     accumulation of hi+mid+lo is exact.
  4. VectorE max-reduces the bank into one y column.

All engines run concurrently across 8 independent bank chains; the
kernel is DVE-reduce-bound (~0.7us x 512 pairs per core).
"""

import sys

sys.path.insert(0, "/opt/trn_rl_repo")

import ml_dtypes
import numpy as np

import concourse.bass as bass
import concourse.mybir as mybir
import concourse.tile as tile
from concourse import bacc
from concourse.bass_utils import run_bass_kernel_spmd

F32 = mybir.dt.float32
BF16 = mybir.dt.bfloat16

B = 1024  # batch
J = 512  # in_features
O = 512  # out_features
N_CORES = 8
O_SH = O // N_CORES  # 64 output features per core
NBT = B // 128  # 8 batch tiles
RPP = 32  # a'-rows per partition group (2 groups at base partitions 0/32)

TRACE = False
LAST_RESULTS = None
_nc_cache = None


def _build_bass(reps: int = 1, loop_reps: int = 1):
    nc = bacc.Bacc("TRN2", target_bir_lowering=False, debug=False, num_devices=N_CORES)
    x_t = nc.dram_tensor("x", [B, J], F32, kind="ExternalInput")
    # 6 rows: [hi|mid|lo] planes for feature rows 0-31, then rows 32-63
    a3_t = nc.dram_tensor("a3", [6, RPP * J], BF16, kind="ExternalInput")
    cs_t = nc.dram_tensor("consts", [3, 128 + J], BF16, kind="ExternalInput")
    y_t = nc.dram_tensor("y", [B, O_SH], F32, kind="ExternalOutput")

    with tile.TileContext(nc) as tc:
        with (
            tc.tile_pool(name="sb", bufs=1) as sb,
            tc.tile_pool(name="ps", bufs=1, space="PSUM") as ps,
        ):
            x_sb = [
                sb.tile([128, J], F32, tag=f"x{t}", name=f"x_sb{t}") for t in range(NBT)
            ]
            for t in range(NBT):
                nc.sync.dma_start(x_sb[t][:], x_t.ap()[t * 128 : (t + 1) * 128, :])
            # plane triples live at base partitions 0 and 32 (the only
            # slice bases bass allows for matmul operands)
            a3_sb = sb.tile([35, RPP * J], BF16)
            nc.sync.dma_start(a3_sb[0:3, :], a3_t.ap()[0:3, :])
            nc.sync.dma_start(a3_sb[32:35, :], a3_t.ap()[3:6, :])
            cs_sb = sb.tile([35, 128 + J], BF16)
            nc.sync.dma_start(cs_sb[0:3, :], cs_t.ap())
            nc.sync.dma_start(cs_sb[32:35, :], cs_t.ap())

            banks = [
                ps.tile([128, J], F32, tag=f"bank{t}", name=f"bank{t}")
                for t in range(NBT)
            ]
            y_sb = [
                sb.tile([128, O_SH], F32, tag=f"y{t}", name=f"y_sb{t}")
                for t in range(NBT)
            ]

            ones0 = cs_sb[0:1, 0:128]
            zeros0 = cs_sb[0:1, 128 : 128 + J]
            # one-time: set has_written for every bank
            for t in range(NBT):
                nc.tensor.matmul(
                    banks[t][:], lhsT=ones0, rhs=zeros0, start=True, stop=False
                )

            def body():
                for i in range(O_SH * reps):
                    i = i % O_SH
                    part = (i // RPP) * 32
                    off = (i % RPP) * J
                    lhsT3 = cs_sb[part : part + 3, 0:128]
                    rhs3 = a3_sb[part : part + 3, off : off + J]
                    for t in range(NBT):
                        nc.scalar.copy(banks[t][:], x_sb[t][:])
                        nc.tensor.matmul(
                            banks[t][:], lhsT=lhsT3, rhs=rhs3, start=False, stop=True
                        )
                        nc.vector.tensor_reduce(
                            y_sb[t][:, i : i + 1],
                            banks[t][:],
                            mybir.AxisListType.X,
                            mybir.AluOpType.max,
                        )

            if loop_reps > 1:
                with tc.For_i(0, loop_reps, 1):
                    body()
            else:
                body()

            for t in range(NBT):
                nc.sync.dma_start(y_t.ap()[t * 128 : (t + 1) * 128, :], y_sb[t][:])
    nc.compile()
    return nc


def _prep_inputs(x, a, bias):
    """Host-side prep: fold bias, exact 3-way bf16 split, per-core shards."""
    a_p = (a.astype(np.float64) + bias.astype(np.float64)[:, None]).astype(np.float32)
    a_hi = a_p.astype(ml_dtypes.bfloat16)
    r1 = a_p - a_hi.astype(np.float32)
    a_mid = r1.astype(ml_dtypes.bfloat16)
    r2 = r1 - a_mid.astype(np.float32)
    a_lo = r2.astype(ml_dtypes.bfloat16)
    assert np.all(r2 - a_lo.astype(np.float32) == 0.0), "bf16 split not exact"

    consts = np.zeros((3, 128 + J), ml_dtypes.bfloat16)
    consts[:, 0:128] = 1.0

    in_maps = []
    for c in range(N_CORES):
        sl = slice(c * O_SH, (c + 1) * O_SH)
        a3 = np.zeros((6, RPP * J), ml_dtypes.bfloat16)
        for g in range(2):  # feature-row group within the shard
            rows = slice(c * O_SH + g * RPP, c * O_SH + (g + 1) * RPP)
            a3[3 * g + 0] = a_hi[rows].reshape(-1)
            a3[3 * g + 1] = a_mid[rows].reshape(-1)
            a3[3 * g + 2] = a_lo[rows].reshape(-1)
        in_maps.append({"x": x, "a3": a3, "consts": consts})
    return in_maps


def kernel(x, a, bias):
    global _nc_cache, LAST_RESULTS
    x = np.ascontiguousarray(np.asarray(x, dtype=np.float32))
    a = np.asarray(a, dtype=np.float32)
    bias = np.asarray(bias, dtype=np.float32)
    assert x.shape == (B, J) and a.shape == (O, J) and bias.shape == (O,)

    if _nc_cache is None:
        _nc_cache = _build_bass()
    nc = _nc_cache

    in_maps = _prep_inputs(x, a, bias)
    res = run_bass_kernel_spmd(nc, in_maps, core_ids=list(range(N_CORES)), trace=TRACE)
    LAST_RESULTS = res
    y = np.concatenate([res.results[c]["y"] for c in range(N_CORES)], axis=1)
    return y
